# revision 22
# baseline (speedup 1.0000x reference)
"""Trainium2 Bass kernel for nn_AutoGraderPrototypeModel (retrieval_knn).

Computes, for full inputs hidden_states [1024, 256, 1024] f32 and
prototype_weight [512, 1024] f32:

    a      = mean(hidden_states, axis=1)                  # [B, D]
    logits = 2 a @ proto.T - ||a||^2 - ||proto||^2        # [B, 512]
    out    = logits.reshape(B, 64, 8).mean(axis=1)        # [B, 8]

Sharding: data-parallel over batch across 8 NeuronCores (128 batch rows
per core, prototype table replicated). The dominant cost is streaming the
128 MiB hidden_states shard from HBM.

DMA layout: strided partition reads (1 MiB partition stride) only reach
~190 GB/s/core on this part, while fully-linear reads reach ~350 GB/s.
Tiles are therefore loaded as flat contiguous [128, WPP] blocks. With
WPP words per partition, partition p of tile i holds WPP/1024 consecutive
t-rows; pooling reduces over t in up to two stages:
  stage 1 (only if WPP > 1024): DVE adds fold the in-partition t-rows;
  stage 2: a PE matmul with a sliding block-column mask (value 1/T)
  scatter-accumulates each batch's partitions into PSUM a[128b, 1024d].

The per-rep epilogue (a -> logits -> label mean, ~19 us serial) is
software-pipelined against the next rep's HBM stream via For_i_pipelined:
stage 0 streams tiles and pools into a_sb, stage 1 computes logits from
the previous rep's a_sb while stage 0's DMA stream saturates HBM.

Measured per-rep device time (robust interleaved multi-rep-count slope;
the axon dispatch quantum drifts 60-130 ms run to run, so 2-point slopes
are unreliable by up to 2x):
  - plain For_i loop (all-engine barrier per rep serializes the ~19 us
    epilogue against the stream): ~413 us
  - 2-stage pipelined loop (this file's default): ~381-395 us
    = ~95-98% of the 358 GB/s per-core HBM roofline (374.9 us).
Rejected by measurement: bulk DMA via the SWDGE/gpsimd path (serializes
against HWDGE transfers), wpp=4096 2 MiB tiles (~399-405 us even with
hs_bufs=6), split_dma half-tiles (~630 us), raw PE pooling without the
DVE fold (PE matmul-bound, ~434 us), hs_bufs=8 / part_bufs=6 /
ring-contiguous halves (all neutral), lag-2 manual pipeline with 4-rep
For_i body (~390 us, so the per-rep barrier is not the residual cost),
a_ps PSUM double-buffering via psum_bufs=2 (~400 us), out DMA on a
HWDGE ring (~400 us; keep it on gpsimd). The cost-model timeline of the
pipelined steady state shows only ~1 us of DMA idle per rep boundary;
the residual HW gap is per-DMA/ring overhead the model underrates.
"""

import os

os.environ.setdefault("JAX_PLATFORMS", "axon,cpu")

from contextlib import ExitStack

import numpy as np

B, T, D = 1024, 256, 1024
M_PROTO = 512
NUM_LABELS = 8
NUM_PROTOTYPES = 64
N_CORES = 8
BS = B // N_CORES  # 128 batch rows per core
P = 128            # SBUF partitions
WPP = 2048         # words per partition per DMA tile (tile = P*WPP*4 bytes)
HS_BUFS = 6

_cached = {}


def _build_program(reps=1, wpp=WPP, hs_bufs=HS_BUFS, act_pt2=False,
                   ttr_asq=False, stage1=True, split_dma=False,
                   pipelined="lag2", unroll=1, dma_pat="sa", fold_pat="v",
                   ring_halves=False, out_eng="g", psum_bufs=1,
                   part_bufs=3, body_ticks=8):
    import concourse.mybir as mybir
    import concourse.tile as tile
    from concourse import bacc, masks

    f32 = mybir.dt.float32
    KD = D // P                      # 8 contraction chunks of 128 over D
    MG = M_PROTO // P                # 4 prototype groups of 128
    words_per_tile = P * wpp
    NT = (BS * T * D) // words_per_tile  # linear tiles per shard
    n_rows = wpp // D                # t-rows per partition (stage-1 depth)
    assert wpp % D == 0
    # batches per tile as a fraction: bpt_num/bpt_den
    bpt_num, bpt_den = words_per_tile, T * D
    n_cols = max(bpt_num // bpt_den, 1)   # mask columns per tile
    grp = P // n_cols if bpt_num >= bpt_den else P

    nc = bacc.Bacc("TRN2", target_bir_lowering=False, debug=False,
                   num_devices=N_CORES)
    hs = nc.dram_tensor("hidden_states", [BS, T, D], f32, kind="ExternalInput").ap()
    pw = nc.dram_tensor("prototype_weight", [M_PROTO, D], f32, kind="ExternalInput").ap()
    out = nc.dram_tensor("out", [BS, NUM_LABELS], f32, kind="ExternalOutput").ap()

    hs_flat = hs.rearrange("b t d -> (b t d)")

    with tile.TileContext(nc) as tc, ExitStack() as ctx:
        hs_pool = ctx.enter_context(tc.tile_pool(name="hs", bufs=hs_bufs))
        part_pool = ctx.enter_context(tc.tile_pool(name="part", bufs=part_bufs))
        work = ctx.enter_context(tc.tile_pool(name="work", bufs=1))
        psum_t = ctx.enter_context(tc.tile_pool(name="psum_t", bufs=2, space="PSUM"))
        psum_a = ctx.enter_context(
            tc.tile_pool(name="psum_a", bufs=1, space="PSUM"))

        state = {}

        def prep():
            ident = work.tile([P, P], f32, tag="ident", name="ident")
            masks.make_identity(nc, ident[:])
            ones_m1 = work.tile([P, 1], f32, tag="ones_m1", name="ones_m1")
            nc.gpsimd.memset(ones_m1[:], 1.0)
            ones_k1 = work.tile([1, P], f32, tag="ones_k1", name="ones_k1")
            nc.gpsimd.memset(ones_k1[:], 1.0)

            # Sliding mask for stage-2 pooling: zp[p, P + c] = 1/T iff
            # c == p // grp (c < n_cols). lhsT for tile i is
            # zp[:, P - s_i : 2P - s_i] with s_i = floor(i * bpt).
            zp = work.tile([P, 2 * P], f32, tag="zp", name="zp")
            nc.gpsimd.memset(zp[:], 0.0)
            for c in range(n_cols):
                nc.gpsimd.memset(zp[grp * c:grp * (c + 1), P + c:P + c + 1],
                                 1.0 / T)

            # protoT2[k] = 2 * proto.T d-chunk; sqT[k] = (2 proto.T)^2
            proto_sb = []
            for j in range(MG):
                pj = work.tile([P, D], f32, tag=f"proto{j}", name=f"proto{j}")
                nc.gpsimd.dma_start(pj[:], pw[j * P:(j + 1) * P, :])
                proto_sb.append(pj)

            protoT2 = [work.tile([P, M_PROTO], f32, tag=f"pT2_{k}", name=f"pT2_{k}")
                       for k in range(KD)]
            sqT = [work.tile([P, M_PROTO], f32, tag=f"sqT_{k}", name=f"sqT_{k}")
                   for k in range(KD)]
            for k in range(KD):
                for j in range(MG):
                    pt = psum_t.tile([P, P], f32, tag="tp", name="pt")
                    nc.tensor.transpose(pt[:], proto_sb[j][:, k * P:(k + 1) * P],
                                        ident[:])
                    if act_pt2:
                        nc.scalar.mul(protoT2[k][:, j * P:(j + 1) * P],
                                      pt[:], 2.0)
                    else:
                        nc.vector.tensor_scalar_mul(
                            protoT2[k][:, j * P:(j + 1) * P], pt[:], 2.0)
                # (2 protoT)^2 = 4 protoT^2; compensated below via -0.25 scale
                nc.vector.tensor_mul(sqT[k][:], protoT2[k][:], protoT2[k][:])

            # b_sq[m] as a [1, 512] row via ones-matmul over squared protoT
            bsq_ps = psum_a.tile([1, M_PROTO], f32, tag="bsq", name="bsq_ps")
            for k in range(KD):
                nc.tensor.matmul(bsq_ps[:], ones_m1[:], sqT[k][:],
                                 start=(k == 0), stop=(k == KD - 1))
            neg_bsq = work.tile([1, M_PROTO], f32, tag="neg_bsq", name="neg_bsq")
            nc.scalar.mul(neg_bsq[:], bsq_ps[:], -0.25)

            state.update(ident=ident, ones_k1=ones_k1, zp=zp, neg_bsq=neg_bsq,
                         protoT2=protoT2)

        def stream_into(a_sb):
            """Stage 0: stream the 128 MiB shard, pool into a_sb [128b, D]."""
            zp = state["zp"]

            # --- pooling: a[b, d] = (1/T) sum_t hs[b, t, d], in PSUM
            a_ps = psum_a.tile([P, D], f32, tag="a_ps", name="a_ps",
                              bufs=psum_bufs)
            eng_map = {"s": nc.sync, "a": nc.scalar, "g": nc.gpsimd,
                       "v": nc.vector, "t": nc.tensor}
            dma_eng = [eng_map[c] for c in dma_pat]
            fold_eng = [eng_map[c] for c in fold_pat]
            if ring_halves:
                # each HWDGE ring walks a contiguous 64 MiB half of the
                # shard (sequential HBM addresses per ring) instead of
                # alternating 1 MiB-strided chunks
                order = [t for k in range(NT // 2) for t in (k, NT // 2 + k)]
            else:
                order = list(range(NT))
            for pos, it in enumerate(order):
                tl = hs_pool.tile([P, wpp], f32, tag="hs", name="tl")
                src = hs_flat[it * words_per_tile:(it + 1) * words_per_tile]
                s2 = src.rearrange("(p w) -> p w", p=P)
                if split_dma:
                    # both HWDGE rings busy every tile: each ring moves a
                    # contiguous half (partition-split keeps linearity)
                    nc.sync.dma_start(tl[0:P // 2, :], s2[0:P // 2, :])
                    nc.scalar.dma_start(tl[P // 2:P, :], s2[P // 2:P, :])
                elif ring_halves:
                    dma_eng[it * len(dma_eng) // NT].dma_start(tl[:], s2)
                else:
                    dma_eng[pos % len(dma_eng)].dma_start(tl[:], s2)
                s_i = (it * bpt_num) // bpt_den
                lhsT = zp[:, P - s_i:2 * P - s_i]
                if stage1 and n_rows > 1:
                    fe = fold_eng[pos % len(fold_eng)]
                    partial = part_pool.tile([P, D], f32, tag="part",
                                             name="partial")
                    fe.tensor_add(partial[:], tl[:, 0:D], tl[:, D:2 * D])
                    for j in range(2, n_rows):
                        fe.tensor_add(partial[:], partial[:],
                                      tl[:, j * D:(j + 1) * D])
                    for h in range(2):
                        nc.tensor.matmul(a_ps[:, h * 512:(h + 1) * 512], lhsT,
                                         partial[:, h * 512:(h + 1) * 512],
                                         start=(it == 0), stop=(it == NT - 1),
                                         skip_group_check=True)
                else:
                    # PE consumes raw t-rows directly; all rows of a tile
                    # share the same mask column (same batch coverage)
                    for r in range(n_rows):
                        for h in range(2):
                            nc.tensor.matmul(
                                a_ps[:, h * 512:(h + 1) * 512], lhsT,
                                tl[:, r * D + h * 512:r * D + (h + 1) * 512],
                                start=(it == 0 and r == 0),
                                stop=(it == NT - 1 and r == n_rows - 1),
                                skip_group_check=True)

            nc.scalar.mul(a_sb[:], a_ps[:], 1.0)

        def epilogue_from(a_sb):
            """Stage 1: a_sb -> logits -> label means -> out DMA."""
            import concourse.mybir as mybir

            ident = state["ident"]
            protoT2 = state["protoT2"]

            # a_sq[b] = sum_d a^2 as per-partition scalar [128, 1]
            sq_tmp = work.tile([P, D], f32, tag="sq_tmp", name="sq_tmp")
            asq = work.tile([P, 1], f32, tag="asq", name="asq")
            if ttr_asq:
                nc.vector.tensor_tensor_reduce(
                    out=sq_tmp[:], in0=a_sb[:], in1=a_sb[:], scale=1.0,
                    scalar=0.0, op0=mybir.AluOpType.mult,
                    op1=mybir.AluOpType.add, accum_out=asq[:])
            else:
                nc.vector.tensor_mul(sq_tmp[:], a_sb[:], a_sb[:])
                nc.vector.tensor_reduce(asq[:], sq_tmp[:],
                                        axis=mybir.AxisListType.X,
                                        op=mybir.AluOpType.add)

            # aT[k] = a.T d-chunk [128d, 128b]
            aTs = []
            for k in range(KD):
                pt = psum_t.tile([P, P], f32, tag="tp", name="pt")
                nc.tensor.transpose(pt[:], a_sb[:, k * P:(k + 1) * P], ident[:])
                aT = work.tile([P, P], f32, tag=f"aT{k}", name=f"aT{k}")
                nc.vector.tensor_copy(aT[:], pt[:])
                aTs.append(aT)

            # logits_pre[b, m] = 2 a@proto.T - b_sq in one PSUM bank
            lg_ps = psum_a.tile([P, M_PROTO], f32, tag="lg", name="lg_ps")
            for k in range(KD):
                nc.tensor.matmul(lg_ps[:], aTs[k][:], protoT2[k][:],
                                 start=(k == 0), stop=False)
            nc.tensor.matmul(lg_ps[:], state["ones_k1"][:], state["neg_bsq"][:],
                             start=False, stop=True)

            # subtract a_sq (per-partition scalar broadcast along free dim)
            lg_sb = work.tile([P, M_PROTO], f32, tag="lg_sb", name="lg_sb")
            nc.vector.tensor_scalar_sub(lg_sb[:], lg_ps[:], asq[:])

            # label mean: out[b, l] = mean_p logits_pre[b, p*8 + l]
            out_sb = work.tile([P, NUM_LABELS], f32, tag="out_sb", name="out_sb")
            lgv = lg_sb[:].rearrange("b (p l) -> b l p", l=NUM_LABELS)
            nc.vector.tensor_reduce(out_sb[:], lgv, axis=mybir.AxisListType.X,
                                    op=mybir.AluOpType.add)
            nc.scalar.mul(out_sb[:], out_sb[:], 1.0 / NUM_PROTOTYPES)
            oe = {"g": nc.gpsimd, "s": nc.sync, "a": nc.scalar}[out_eng]
            oe.dma_start(out[:, :], out_sb[:])

        prep()

        import concourse.mybir as mybir
        hints = (mybir.EngineType.DVE, mybir.EngineType.PE,
                 mybir.EngineType.Activation, mybir.EngineType.SP,
                 mybir.EngineType.Pool)

        if pipelined == "lag2" and reps >= 6:
            # Manual lag-2 software pipeline: 4 ping-pong a_sb buffers, a
            # plain For_i whose body covers 4 reps, each tick = epilogue of
            # the rep 2 back + stream of the current rep. The lag-2 distance
            # keeps every in-body epilogue's input 2 full streams old, so no
            # engine queue blocks on a same-body stream, and the For_i
            # all-engine barrier is paid once per 4 reps instead of per rep.
            assert body_ticks % 4 == 0
            asb = [work.tile([P, D], f32, tag=f"asb{j}", name=f"asb{j}")
                   for j in range(4)]
            K_body = (reps - 2) // body_ticks
            r_rem = (reps - 2) % body_ticks
            stream_into(asb[0])
            stream_into(asb[1])
            with tc.For_i(0, K_body, 1, hint_engines=hints):
                for jt in range(body_ticks):
                    epilogue_from(asb[jt % 4])
                    stream_into(asb[(jt + 2) % 4])
            j = 0
            for _ in range(r_rem):
                epilogue_from(asb[j])
                stream_into(asb[(j + 2) % 4])
                j = (j + 1) % 4
            epilogue_from(asb[j])
            epilogue_from(asb[(j + 1) % 4])
        elif pipelined == "py" or pipelined == "lag2":
            # Python-unrolled 2-stage pipeline (no HW loop) — same emission
            # order as For_i_pipelined (stage 1 of i-1 before stage 0 of i).
            # Used for TimelineSim diagnosis; IR size grows with reps.
            asb = [work.tile([P, D], f32, tag=f"asb{j}", name=f"asb{j}")
                   for j in range(2)]
            stream_into(asb[0])
            for i in range(1, reps):
                epilogue_from(asb[(i - 1) % 2])
                stream_into(asb[i % 2])
            epilogue_from(asb[(reps - 1) % 2])
        elif pipelined:
            def s0(pipe, iv):
                a_sb = pipe.intermediate_tile([P, D], f32, name="a_sb")
                stream_into(a_sb)
                return a_sb

            def s1(pipe, iv, a_sb):
                epilogue_from(a_sb)

            tc.For_i_pipelined([s0, s1], 0, reps, unroll=unroll,
                               hint_engines=hints)
        else:
            def body():
                a_sb = work.tile([P, D], f32, tag="a", name="a_sb")
                stream_into(a_sb)
                epilogue_from(a_sb)

            if reps == 1:
                body()
            else:
                with tc.For_i(0, reps, 1, hint_engines=hints):
                    body()

    nc.compile()
    return nc


def _get_program(reps=1, **kw):
    key = (reps, tuple(sorted(kw.items())))
    if key not in _cached:
        _cached[key] = _build_program(reps, **kw)
    return _cached[key]


def _make_in_maps(hs, pw):
    return [
        {
            "hidden_states": np.ascontiguousarray(hs[i * BS:(i + 1) * BS]),
            "prototype_weight": pw,
        }
        for i in range(N_CORES)
    ]


def run(hidden_states, prototype_weight, trace=False, reps=1, **kw):
    """Run the SPMD kernel; returns (full_output, BassKernelResults)."""
    from concourse.bass_utils import run_bass_kernel_spmd

    hs = np.ascontiguousarray(np.asarray(hidden_states, dtype=np.float32))
    pw = np.ascontiguousarray(np.asarray(prototype_weight, dtype=np.float32))
    assert hs.shape == (B, T, D), hs.shape
    assert pw.shape == (M_PROTO, D), pw.shape

    nc = _get_program(reps, **kw)
    res = run_bass_kernel_spmd(nc, _make_in_maps(hs, pw),
                               core_ids=list(range(N_CORES)), trace=trace)
    full = np.concatenate([res.results[i]["out"] for i in range(N_CORES)], axis=0)
    return full, res


def kernel(hidden_states, prototype_weight):
    full, _ = run(hidden_states, prototype_weight, trace=False)
    return full


# revision 23
# speedup vs baseline: 1.0273x; 1.0273x over previous
"""Trainium2 Bass kernel for nn_AutoGraderPrototypeModel (retrieval_knn).

Computes, for full inputs hidden_states [1024, 256, 1024] f32 and
prototype_weight [512, 1024] f32:

    a      = mean(hidden_states, axis=1)                  # [B, D]
    logits = 2 a @ proto.T - ||a||^2 - ||proto||^2        # [B, 512]
    out    = logits.reshape(B, 64, 8).mean(axis=1)        # [B, 8]

Sharding: data-parallel over batch across 8 NeuronCores (128 batch rows
per core, prototype table replicated). The dominant cost is streaming the
128 MiB hidden_states shard from HBM.

DMA layout: strided partition reads (1 MiB partition stride) only reach
~190 GB/s/core on this part, while fully-linear reads reach ~350 GB/s.
Tiles are therefore loaded as flat contiguous [128, WPP] blocks. With
WPP words per partition, partition p of tile i holds WPP/1024 consecutive
t-rows; pooling reduces over t in up to two stages:
  stage 1 (only if WPP > 1024): DVE adds fold the in-partition t-rows;
  stage 2: a PE matmul with a sliding block-column mask (value 1/T)
  scatter-accumulates each batch's partitions into PSUM a[128b, 1024d].

The per-rep epilogue (a -> logits -> label mean, ~19 us serial) is
software-pipelined against the next rep's HBM stream via For_i_pipelined:
stage 0 streams tiles and pools into a_sb, stage 1 computes logits from
the previous rep's a_sb while stage 0's DMA stream saturates HBM.

Measured per-rep device time (robust interleaved multi-rep-count slope;
the axon dispatch quantum drifts 60-130 ms run to run, so 2-point slopes
are unreliable by up to 2x):
  - plain For_i loop (all-engine barrier per rep serializes the ~19 us
    epilogue against the stream): ~413 us
  - 2-stage pipelined loop (this file's default): ~381-395 us
    = ~95-98% of the 358 GB/s per-core HBM roofline (374.9 us).
Rejected by measurement: bulk DMA via the SWDGE/gpsimd path (serializes
against HWDGE transfers), wpp=4096 2 MiB tiles (~399-405 us even with
hs_bufs=6), split_dma half-tiles (~630 us), raw PE pooling without the
DVE fold (PE matmul-bound, ~434 us), hs_bufs=8 / part_bufs=6 /
ring-contiguous halves (all neutral), lag-2 manual pipeline with 4-rep
For_i body (~390 us) and 8-rep body (384-397 us across two runs, both
within the default's noise band, so the per-rep barrier is cheap),
a_ps PSUM double-buffering via psum_bufs=2 (~400 us), out DMA on a
HWDGE ring (~400 us; keep it on gpsimd). The cost-model timeline of the
pipelined steady state shows only ~1 us of DMA idle per rep boundary;
the residual HW gap is per-DMA/ring overhead the model underrates.
"""

import os

os.environ.setdefault("JAX_PLATFORMS", "axon,cpu")

from contextlib import ExitStack

import numpy as np

B, T, D = 1024, 256, 1024
M_PROTO = 512
NUM_LABELS = 8
NUM_PROTOTYPES = 64
N_CORES = 8
BS = B // N_CORES  # 128 batch rows per core
P = 128            # SBUF partitions
WPP = 2048         # words per partition per DMA tile (tile = P*WPP*4 bytes)
HS_BUFS = 6

_cached = {}


def _build_program(reps=1, wpp=WPP, hs_bufs=HS_BUFS, act_pt2=False,
                   ttr_asq=False, stage1=True, split_dma=False,
                   pipelined=True, unroll=1, dma_pat="sa", fold_pat="v",
                   ring_halves=False, out_eng="g", psum_bufs=1,
                   part_bufs=3, body_ticks=8):
    import concourse.mybir as mybir
    import concourse.tile as tile
    from concourse import bacc, masks

    f32 = mybir.dt.float32
    KD = D // P                      # 8 contraction chunks of 128 over D
    MG = M_PROTO // P                # 4 prototype groups of 128
    words_per_tile = P * wpp
    NT = (BS * T * D) // words_per_tile  # linear tiles per shard
    n_rows = wpp // D                # t-rows per partition (stage-1 depth)
    assert wpp % D == 0
    # batches per tile as a fraction: bpt_num/bpt_den
    bpt_num, bpt_den = words_per_tile, T * D
    n_cols = max(bpt_num // bpt_den, 1)   # mask columns per tile
    grp = P // n_cols if bpt_num >= bpt_den else P

    nc = bacc.Bacc("TRN2", target_bir_lowering=False, debug=False,
                   num_devices=N_CORES)
    hs = nc.dram_tensor("hidden_states", [BS, T, D], f32, kind="ExternalInput").ap()
    pw = nc.dram_tensor("prototype_weight", [M_PROTO, D], f32, kind="ExternalInput").ap()
    out = nc.dram_tensor("out", [BS, NUM_LABELS], f32, kind="ExternalOutput").ap()

    hs_flat = hs.rearrange("b t d -> (b t d)")

    with tile.TileContext(nc) as tc, ExitStack() as ctx:
        hs_pool = ctx.enter_context(tc.tile_pool(name="hs", bufs=hs_bufs))
        part_pool = ctx.enter_context(tc.tile_pool(name="part", bufs=part_bufs))
        work = ctx.enter_context(tc.tile_pool(name="work", bufs=1))
        psum_t = ctx.enter_context(tc.tile_pool(name="psum_t", bufs=2, space="PSUM"))
        psum_a = ctx.enter_context(
            tc.tile_pool(name="psum_a", bufs=1, space="PSUM"))

        state = {}

        def prep():
            ident = work.tile([P, P], f32, tag="ident", name="ident")
            masks.make_identity(nc, ident[:])
            ones_m1 = work.tile([P, 1], f32, tag="ones_m1", name="ones_m1")
            nc.gpsimd.memset(ones_m1[:], 1.0)
            ones_k1 = work.tile([1, P], f32, tag="ones_k1", name="ones_k1")
            nc.gpsimd.memset(ones_k1[:], 1.0)

            # Sliding mask for stage-2 pooling: zp[p, P + c] = 1/T iff
            # c == p // grp (c < n_cols). lhsT for tile i is
            # zp[:, P - s_i : 2P - s_i] with s_i = floor(i * bpt).
            zp = work.tile([P, 2 * P], f32, tag="zp", name="zp")
            nc.gpsimd.memset(zp[:], 0.0)
            for c in range(n_cols):
                nc.gpsimd.memset(zp[grp * c:grp * (c + 1), P + c:P + c + 1],
                                 1.0 / T)

            # protoT2[k] = 2 * proto.T d-chunk; sqT[k] = (2 proto.T)^2
            proto_sb = []
            for j in range(MG):
                pj = work.tile([P, D], f32, tag=f"proto{j}", name=f"proto{j}")
                nc.gpsimd.dma_start(pj[:], pw[j * P:(j + 1) * P, :])
                proto_sb.append(pj)

            protoT2 = [work.tile([P, M_PROTO], f32, tag=f"pT2_{k}", name=f"pT2_{k}")
                       for k in range(KD)]
            sqT = [work.tile([P, M_PROTO], f32, tag=f"sqT_{k}", name=f"sqT_{k}")
                   for k in range(KD)]
            for k in range(KD):
                for j in range(MG):
                    pt = psum_t.tile([P, P], f32, tag="tp", name="pt")
                    nc.tensor.transpose(pt[:], proto_sb[j][:, k * P:(k + 1) * P],
                                        ident[:])
                    if act_pt2:
                        nc.scalar.mul(protoT2[k][:, j * P:(j + 1) * P],
                                      pt[:], 2.0)
                    else:
                        nc.vector.tensor_scalar_mul(
                            protoT2[k][:, j * P:(j + 1) * P], pt[:], 2.0)
                # (2 protoT)^2 = 4 protoT^2; compensated below via -0.25 scale
                nc.vector.tensor_mul(sqT[k][:], protoT2[k][:], protoT2[k][:])

            # b_sq[m] as a [1, 512] row via ones-matmul over squared protoT
            bsq_ps = psum_a.tile([1, M_PROTO], f32, tag="bsq", name="bsq_ps")
            for k in range(KD):
                nc.tensor.matmul(bsq_ps[:], ones_m1[:], sqT[k][:],
                                 start=(k == 0), stop=(k == KD - 1))
            neg_bsq = work.tile([1, M_PROTO], f32, tag="neg_bsq", name="neg_bsq")
            nc.scalar.mul(neg_bsq[:], bsq_ps[:], -0.25)

            state.update(ident=ident, ones_k1=ones_k1, zp=zp, neg_bsq=neg_bsq,
                         protoT2=protoT2)

        def stream_into(a_sb):
            """Stage 0: stream the 128 MiB shard, pool into a_sb [128b, D]."""
            zp = state["zp"]

            # --- pooling: a[b, d] = (1/T) sum_t hs[b, t, d], in PSUM
            a_ps = psum_a.tile([P, D], f32, tag="a_ps", name="a_ps",
                              bufs=psum_bufs)
            eng_map = {"s": nc.sync, "a": nc.scalar, "g": nc.gpsimd,
                       "v": nc.vector, "t": nc.tensor}
            dma_eng = [eng_map[c] for c in dma_pat]
            fold_eng = [eng_map[c] for c in fold_pat]
            if ring_halves:
                # each HWDGE ring walks a contiguous 64 MiB half of the
                # shard (sequential HBM addresses per ring) instead of
                # alternating 1 MiB-strided chunks
                order = [t for k in range(NT // 2) for t in (k, NT // 2 + k)]
            else:
                order = list(range(NT))
            for pos, it in enumerate(order):
                tl = hs_pool.tile([P, wpp], f32, tag="hs", name="tl")
                src = hs_flat[it * words_per_tile:(it + 1) * words_per_tile]
                s2 = src.rearrange("(p w) -> p w", p=P)
                if split_dma:
                    # both HWDGE rings busy every tile: each ring moves a
                    # contiguous half (partition-split keeps linearity)
                    nc.sync.dma_start(tl[0:P // 2, :], s2[0:P // 2, :])
                    nc.scalar.dma_start(tl[P // 2:P, :], s2[P // 2:P, :])
                elif ring_halves:
                    dma_eng[it * len(dma_eng) // NT].dma_start(tl[:], s2)
                else:
                    dma_eng[pos % len(dma_eng)].dma_start(tl[:], s2)
                s_i = (it * bpt_num) // bpt_den
                lhsT = zp[:, P - s_i:2 * P - s_i]
                if stage1 and n_rows > 1:
                    fe = fold_eng[pos % len(fold_eng)]
                    partial = part_pool.tile([P, D], f32, tag="part",
                                             name="partial")
                    fe.tensor_add(partial[:], tl[:, 0:D], tl[:, D:2 * D])
                    for j in range(2, n_rows):
                        fe.tensor_add(partial[:], partial[:],
                                      tl[:, j * D:(j + 1) * D])
                    for h in range(2):
                        nc.tensor.matmul(a_ps[:, h * 512:(h + 1) * 512], lhsT,
                                         partial[:, h * 512:(h + 1) * 512],
                                         start=(it == 0), stop=(it == NT - 1),
                                         skip_group_check=True)
                else:
                    # PE consumes raw t-rows directly; all rows of a tile
                    # share the same mask column (same batch coverage)
                    for r in range(n_rows):
                        for h in range(2):
                            nc.tensor.matmul(
                                a_ps[:, h * 512:(h + 1) * 512], lhsT,
                                tl[:, r * D + h * 512:r * D + (h + 1) * 512],
                                start=(it == 0 and r == 0),
                                stop=(it == NT - 1 and r == n_rows - 1),
                                skip_group_check=True)

            nc.scalar.mul(a_sb[:], a_ps[:], 1.0)

        def epilogue_from(a_sb):
            """Stage 1: a_sb -> logits -> label means -> out DMA."""
            import concourse.mybir as mybir

            ident = state["ident"]
            protoT2 = state["protoT2"]

            # a_sq[b] = sum_d a^2 as per-partition scalar [128, 1]
            sq_tmp = work.tile([P, D], f32, tag="sq_tmp", name="sq_tmp")
            asq = work.tile([P, 1], f32, tag="asq", name="asq")
            if ttr_asq:
                nc.vector.tensor_tensor_reduce(
                    out=sq_tmp[:], in0=a_sb[:], in1=a_sb[:], scale=1.0,
                    scalar=0.0, op0=mybir.AluOpType.mult,
                    op1=mybir.AluOpType.add, accum_out=asq[:])
            else:
                nc.vector.tensor_mul(sq_tmp[:], a_sb[:], a_sb[:])
                nc.vector.tensor_reduce(asq[:], sq_tmp[:],
                                        axis=mybir.AxisListType.X,
                                        op=mybir.AluOpType.add)

            # aT[k] = a.T d-chunk [128d, 128b]
            aTs = []
            for k in range(KD):
                pt = psum_t.tile([P, P], f32, tag="tp", name="pt")
                nc.tensor.transpose(pt[:], a_sb[:, k * P:(k + 1) * P], ident[:])
                aT = work.tile([P, P], f32, tag=f"aT{k}", name=f"aT{k}")
                nc.vector.tensor_copy(aT[:], pt[:])
                aTs.append(aT)

            # logits_pre[b, m] = 2 a@proto.T - b_sq in one PSUM bank
            lg_ps = psum_a.tile([P, M_PROTO], f32, tag="lg", name="lg_ps")
            for k in range(KD):
                nc.tensor.matmul(lg_ps[:], aTs[k][:], protoT2[k][:],
                                 start=(k == 0), stop=False)
            nc.tensor.matmul(lg_ps[:], state["ones_k1"][:], state["neg_bsq"][:],
                             start=False, stop=True)

            # subtract a_sq (per-partition scalar broadcast along free dim)
            lg_sb = work.tile([P, M_PROTO], f32, tag="lg_sb", name="lg_sb")
            nc.vector.tensor_scalar_sub(lg_sb[:], lg_ps[:], asq[:])

            # label mean: out[b, l] = mean_p logits_pre[b, p*8 + l]
            out_sb = work.tile([P, NUM_LABELS], f32, tag="out_sb", name="out_sb")
            lgv = lg_sb[:].rearrange("b (p l) -> b l p", l=NUM_LABELS)
            nc.vector.tensor_reduce(out_sb[:], lgv, axis=mybir.AxisListType.X,
                                    op=mybir.AluOpType.add)
            nc.scalar.mul(out_sb[:], out_sb[:], 1.0 / NUM_PROTOTYPES)
            oe = {"g": nc.gpsimd, "s": nc.sync, "a": nc.scalar}[out_eng]
            oe.dma_start(out[:, :], out_sb[:])

        prep()

        import concourse.mybir as mybir
        hints = (mybir.EngineType.DVE, mybir.EngineType.PE,
                 mybir.EngineType.Activation, mybir.EngineType.SP,
                 mybir.EngineType.Pool)

        if pipelined == "lag2" and reps >= 6:
            # Manual lag-2 software pipeline: 4 ping-pong a_sb buffers, a
            # plain For_i whose body covers 4 reps, each tick = epilogue of
            # the rep 2 back + stream of the current rep. The lag-2 distance
            # keeps every in-body epilogue's input 2 full streams old, so no
            # engine queue blocks on a same-body stream, and the For_i
            # all-engine barrier is paid once per 4 reps instead of per rep.
            assert body_ticks % 4 == 0
            asb = [work.tile([P, D], f32, tag=f"asb{j}", name=f"asb{j}")
                   for j in range(4)]
            K_body = (reps - 2) // body_ticks
            r_rem = (reps - 2) % body_ticks
            stream_into(asb[0])
            stream_into(asb[1])
            with tc.For_i(0, K_body, 1, hint_engines=hints):
                for jt in range(body_ticks):
                    epilogue_from(asb[jt % 4])
                    stream_into(asb[(jt + 2) % 4])
            j = 0
            for _ in range(r_rem):
                epilogue_from(asb[j])
                stream_into(asb[(j + 2) % 4])
                j = (j + 1) % 4
            epilogue_from(asb[j])
            epilogue_from(asb[(j + 1) % 4])
        elif pipelined == "py" or pipelined == "lag2":
            # Python-unrolled 2-stage pipeline (no HW loop) — same emission
            # order as For_i_pipelined (stage 1 of i-1 before stage 0 of i).
            # Used for TimelineSim diagnosis; IR size grows with reps.
            asb = [work.tile([P, D], f32, tag=f"asb{j}", name=f"asb{j}")
                   for j in range(2)]
            stream_into(asb[0])
            for i in range(1, reps):
                epilogue_from(asb[(i - 1) % 2])
                stream_into(asb[i % 2])
            epilogue_from(asb[(reps - 1) % 2])
        elif pipelined:
            def s0(pipe, iv):
                a_sb = pipe.intermediate_tile([P, D], f32, name="a_sb")
                stream_into(a_sb)
                return a_sb

            def s1(pipe, iv, a_sb):
                epilogue_from(a_sb)

            tc.For_i_pipelined([s0, s1], 0, reps, unroll=unroll,
                               hint_engines=hints)
        else:
            def body():
                a_sb = work.tile([P, D], f32, tag="a", name="a_sb")
                stream_into(a_sb)
                epilogue_from(a_sb)

            if reps == 1:
                body()
            else:
                with tc.For_i(0, reps, 1, hint_engines=hints):
                    body()

    nc.compile()
    return nc


def _get_program(reps=1, **kw):
    key = (reps, tuple(sorted(kw.items())))
    if key not in _cached:
        _cached[key] = _build_program(reps, **kw)
    return _cached[key]


def _make_in_maps(hs, pw):
    return [
        {
            "hidden_states": np.ascontiguousarray(hs[i * BS:(i + 1) * BS]),
            "prototype_weight": pw,
        }
        for i in range(N_CORES)
    ]


def run(hidden_states, prototype_weight, trace=False, reps=1, **kw):
    """Run the SPMD kernel; returns (full_output, BassKernelResults)."""
    from concourse.bass_utils import run_bass_kernel_spmd

    hs = np.ascontiguousarray(np.asarray(hidden_states, dtype=np.float32))
    pw = np.ascontiguousarray(np.asarray(prototype_weight, dtype=np.float32))
    assert hs.shape == (B, T, D), hs.shape
    assert pw.shape == (M_PROTO, D), pw.shape

    nc = _get_program(reps, **kw)
    res = run_bass_kernel_spmd(nc, _make_in_maps(hs, pw),
                               core_ids=list(range(N_CORES)), trace=trace)
    full = np.concatenate([res.results[i]["out"] for i in range(N_CORES)], axis=0)
    return full, res


def kernel(hidden_states, prototype_weight):
    full, _ = run(hidden_states, prototype_weight, trace=False)
    return full
